# revision 9
# baseline (speedup 1.0000x reference)
"""Cosine-similarity (pairwise, normalized by sqrt(|a||b|)+eps) Trainium2 kernel.

Problem: first_vector [8192, 512] f32, second_vector [8192, 512] f32,
output sim [8192, 8192] f32 with
    sim = (A @ B.T) / (sqrt(|A_n| * |B_m|) + 1e-6)        (normalize=1)

Strategy (8 NeuronCores, SPMD, no collectives):
  * 2D shard: 4-way over A rows x 2-way over B rows. Core c=(ni,mj)
    computes the [2048, 4096] output slab at (ni*2048, mj*4096).
  * fp16 operands (the GEMM runs at the fp16 PE roofline, ~215ns per
    128x128x512 matmul; fp32 is 1/4 rate) and fp16 output stores (upcast
    to f32 on host). The all-f32 baseline was DMA-bound: 46MB at
    ~300GB/s/core = 154us > ~122us of PE work.
  * Inputs are packed host-side into a [*, 128, 1024] tile-pair layout:
    partition p of pair tp holds rows tp*256+p and tp*256+128+p
    back-to-back, i.e. two natural [128, 512] row-tiles side by side.
    This keeps 2KB contiguous per partition line - f16 tiles loaded from
    the row-major layout have 1KB lines, which run the DMA engines at
    half throughput (~190GB/s measured vs ~300+).
  * Normalization sim = (a.b) * |a|^-1/2 * |b|^-1/2 is split:
      - B rows pre-scaled by ssq_b^-1/4 (f16 multiply) before transpose,
      - A's ssq_a^-1/4 applied during PSUM evacuation - free on both
        evacuation engines (ACT Copy takes a per-partition scale operand;
        DVE uses tensor_scalar_mul). A's pipeline stays norm-free:
        DMA -> PE transpose, so the GEMM starts as soon as tiles land.
    The +eps in the reference denominator is dropped (~5e-8 rel).
  * All sums-of-squares run on DVE (tensor_tensor square + tensor_reduce,
    one pair-tile per op) - the ACT-side square+accumulate costs 705+278ns
    a tile, needs an activation-table load, and serialized the B-group
    chains behind evacuation copies in earlier versions.
    (tensor_tensor_reduce would fuse the two, but it crashes the exec
    unit on this silicon - probed.)
  * Transposes to d-major are PE matmuls against an f16 identity (~56ns
    warm, hidden in the GEMM stream); their PSUM->SBUF CASTs go to ACT,
    which is otherwise light. GEMM evacuations alternate DVE/ACT and
    cover 2 PSUM banks per instruction.
  * ACT tables (Copy/Sqrt) are preloaded via dummy ops at t=0 (they
    otherwise lazy-load 1.28us in the middle of the first chain), and a
    few warm-up matmuls run during the input-DMA wait so the PE's HAM
    clock gate is at 8/8 when the real stream begins.
"""

import numpy as np

_N, _M, _D = 8192, 8192, 512
_P = 128
_GRID_N, _GRID_M = 4, 2
_AN = _N // _GRID_N        # A rows per core (2048)
_BM = _M // _GRID_M        # B rows per core (4096)
_KC = _D // _P             # contraction chunks (4)
_NS = 512                  # moving free dim per matmul (one PSUM bank of f32)

TRACE = False              # test harness sets True to collect an NTFF profile
LAST_RESULTS = None        # BassKernelResults of the last run (for test.py)

_NC_CACHE = {}


def _build_nc(normalize: bool):
    import concourse.bass as bass
    import concourse.mybir as mybir
    import concourse.tile as tile
    from concourse import bacc
    from concourse.masks import make_identity

    f32 = mybir.dt.float32
    f16 = mybir.dt.float16
    nc = bacc.Bacc("TRN2", target_bir_lowering=False, debug=False,
                   enable_asserts=False)

    KA = _AN // _P             # 16 A row-tiles (8 pairs)
    NSC = _BM // _NS           # 8 B column groups of 512 (2 pairs each)
    CP = mybir.ActivationFunctionType.Copy
    MUL = mybir.AluOpType.mult
    ADD = mybir.AluOpType.add

    # Tile-pair packed inputs (see module docstring).
    a_d = nc.declare_dram_parameter("a", [_AN // 2, 2 * _D], f16, isOutput=False)
    b_d = nc.declare_dram_parameter("b", [_BM // 2, 2 * _D], f16, isOutput=False)
    out_d = nc.declare_dram_parameter("out", [_AN, _BM], f16, isOutput=True)

    with tile.TileContext(nc) as tc:
        with (
            tc.tile_pool(name="const", bufs=1) as const_pool,
            tc.tile_pool(name="persist", bufs=1) as persist,
            tc.tile_pool(name="natp", bufs=6) as natp,
            tc.tile_pool(name="scaledp", bufs=4) as scaledp,
            tc.tile_pool(name="sqp", bufs=2) as sqp,
            tc.tile_pool(name="scal", bufs=6) as scal,
            tc.tile_pool(name="tpa", bufs=2, space=bass.MemorySpace.PSUM) as tpa,
            tc.tile_pool(name="tpb", bufs=1, space=bass.MemorySpace.PSUM) as tpb,
            tc.tile_pool(name="mpsum", bufs=2, space=bass.MemorySpace.PSUM) as mpsum,
        ):
            # ACT table preloads (Copy -> table 0, Sqrt -> table 1) on
            # dependency-free data, overlapping the first input DMAs.
            dsrc = const_pool.tile([_P, 1], f32)
            nc.vector.memset(dsrc[:], 1.0)
            ddst = const_pool.tile([_P, 1], f32)
            nc.scalar.copy(ddst[:], dsrc[:])
            nc.scalar.sqrt(ddst[:], dsrc[:])

            # Warm-up stream source (no GpSimd dep, unlike the identity).
            wsrc = const_pool.tile([_P, _NS], f16)
            nc.vector.memset(wsrc[:], 0.5)

            ident = const_pool.tile([_P, _P], f16)
            make_identity(nc, ident[:])

            # d-major (transposed) f16 operands, one tile per A row-tile /
            # per B column group so GEMM dependencies stay fine-grained.
            aTt = [persist.tile([_P, _KC * _P], f16, name=f"aT{t}", tag=f"aT{t}")
                   for t in range(KA)]                       # 16 x 128 KB
            bTs = [persist.tile([_P, _KC, _NS], f16, name=f"bS{s}", tag=f"bS{s}")
                   for s in range(NSC)]                      # 8 x 512 KB
            # A row-tile evacuation scales ssq_a^-1/4, one column per tile.
            sA = persist.tile([_P, KA], f32, name="sA", tag="sA")

            # Output staging: 8 slots x 2 row-tiles x 1024 cols (f16).
            ostP = [persist.tile([_P, 2, 2 * _NS], f16, name=f"ost{i}",
                                 tag=f"ost{i}")
                    for i in range(KA // 2)]

            # Warm the PE / HAM clock gate during the input-DMA wait.
            # (tag="pt": pools key buffer slots by tag, which defaults to
            # the assignee name - an own tag would cost an extra bank.)
            warm = tpa.tile([_P, _KC * _P], f32, tag="pt")
            for _ in range(2):
                nc.tensor.matmul(warm[:], lhsT=ident[:], rhs=wsrc[:],
                                 start=True, stop=True)
            warm2 = tpa.tile([_P, _KC * _P], f32, tag="pt")
            for _ in range(2):
                nc.tensor.matmul(warm2[:], lhsT=ident[:], rhs=wsrc[:],
                                 start=True, stop=True)

            def ssq_pair(nat2, ssq2):
                """ssq2[:, 0:2] = row sums-of-squares of a [128, 2, 512]
                f16 tile-pair. Both ops on DVE."""
                sq2 = sqp.tile([_P, 2, _D], f16, tag="sq")
                nc.vector.tensor_tensor(sq2[:], nat2[:], nat2[:], op=MUL)
                nc.vector.tensor_reduce(ssq2, sq2[:],
                                        axis=mybir.AxisListType.X, op=ADD)

            def chain(ssqn, dst):
                """dst = ssq^-1/4 for a [128, nj] tile of row ssq."""
                nj = ssqn.shape[1]
                rec = scal.tile([_P, nj], f32, tag="rec")
                nc.vector.reciprocal(rec[:], ssqn[:])
                sh = scal.tile([_P, nj], f32, tag="sh")
                nc.scalar.sqrt(sh[:], rec[:])
                nc.scalar.sqrt(dst, sh[:])

            def prep_a(g):
                """A tile-pairs 2g, 2g+1 (row-tiles 4g..4g+3) -> aTt[...],
                plus the evacuation-scale chain into sA[:, 4g:4g+4]."""
                nats = []
                ssq4 = (scal.tile([_P, 4], f32, name="ssq4", tag="ssq")
                        if normalize else None)
                for pr in range(2):
                    nat2 = natp.tile([_P, 2, _D], f16, tag="nat")
                    tp = 2 * g + pr
                    nc.sync.dma_start(nat2[:], a_d[tp * _P:(tp + 1) * _P, :]
                                      .rearrange("p (j d) -> p j d", j=2))
                    nats.append(nat2)
                    if normalize:
                        ssq_pair(nat2, ssq4[:, 2 * pr:2 * pr + 2])
                if normalize:
                    chain(ssq4[:], sA[:, 4 * g:4 * g + 4])
                for j in range(4):
                    nat2 = nats[j // 2]
                    pt = tpa.tile([_P, _KC * _P], f32, tag="pt")
                    for k in range(_KC):
                        nc.tensor.matmul(
                            pt[:, k * _P:(k + 1) * _P],
                            lhsT=nat2[:, j % 2, k * _P:(k + 1) * _P],
                            rhs=ident[:],
                            start=True,
                            stop=True,
                        )
                    nc.scalar.copy(aTt[4 * g + j][:], pt[:])

            def prep_b_load(sg):
                """Phase 1 for B column group sg: tile-pair loads and
                sums-of-squares (DVE). Returns state for prep_b_finish."""
                nats = []
                ssq4 = (scal.tile([_P, 4], f32, name="ssq4", tag="ssq")
                        if normalize else None)
                for pr in range(2):
                    nat2 = natp.tile([_P, 2, _D], f16, tag="nat")
                    tp = 2 * sg + pr
                    nc.sync.dma_start(nat2[:], b_d[tp * _P:(tp + 1) * _P, :]
                                      .rearrange("p (j d) -> p j d", j=2))
                    nats.append(nat2)
                    if normalize:
                        ssq_pair(nat2, ssq4[:, 2 * pr:2 * pr + 2])
                return nats, ssq4

            def prep_b_finish(sg, state):
                """Phase 2: scale rows by ssq^-1/4, transpose into bTs[sg]."""
                nats, ssq4 = state
                if normalize:
                    s4 = scal.tile([_P, 4], f32, tag="s4")
                    chain(ssq4[:], s4[:])
                    scl = []
                    for j in range(4):
                        scaled = scaledp.tile([_P, _D], f16, tag="scaled")
                        nc.vector.tensor_scalar_mul(
                            scaled[:], in0=nats[j // 2][:, j % 2],
                            scalar1=s4[:, j:j + 1]
                        )
                        scl.append(lambda k, s=scaled: s[:, k * _P:(k + 1) * _P])
                else:
                    scl = [lambda k, n=nats[j // 2], jj=j % 2: n[:, jj, k * _P:(k + 1) * _P]
                           for j in range(4)]
                for jp in range(2):
                    ptb = tpb.tile([_P, _KC, 2 * _P], f32, tag="ptb")
                    for jj in range(2):
                        for k in range(_KC):
                            nc.tensor.matmul(
                                ptb[:, k, jj * _P:(jj + 1) * _P],
                                lhsT=scl[2 * jp + jj](k),
                                rhs=ident[:],
                                start=True,
                                stop=True,
                            )
                    nc.scalar.copy(
                        bTs[sg][:, :, 2 * jp * _P:2 * (jp + 1) * _P], ptb[:]
                    )

            def prep_b(sg):
                prep_b_finish(sg, prep_b_load(sg))

            # Fast start: shortest chain to the first GEMM matmul, then
            # backfill. DMA queues drain in emission order, so this is
            # also the input-arrival order.
            prep_a(0)
            prep_b(0)
            prep_b(1)
            prep_a(1)
            prep_a(2)
            prep_a(3)

            cidx = 0

            def evac(dst, ps, t):
                """PSUM -> f16 SBUF, folding in A's normalization factor.
                Alternates DVE / ACT."""
                nonlocal cidx
                if normalize:
                    if cidx % 2 == 0:
                        nc.vector.tensor_scalar_mul(dst, in0=ps,
                                                    scalar1=sA[:, t:t + 1])
                    else:
                        nc.scalar.activation(dst, ps, CP,
                                             scale=sA[:, t:t + 1])
                else:
                    if cidx % 2 == 0:
                        nc.vector.tensor_copy(dst, ps)
                    else:
                        nc.scalar.copy(dst, ps)
                cidx += 1

            def mm_ts(t, s, pdst):
                for k in range(_KC):
                    nc.tensor.matmul(
                        pdst,
                        lhsT=aTt[t][:, k * _P:(k + 1) * _P],
                        rhs=bTs[s][:, k, :],
                        start=(k == 0),
                        stop=(k == _KC - 1),
                    )

            # Column-group pairs, t-major: both B groups of the pair per
            # row-tile, one 2-bank evacuation + one 256KB store per
            # (t, pair). Remaining B prep is staggered through the loop,
            # a pair ahead of use.
            for p in range(NSC // 2):
                for t in range(KA):
                    if p == 0:
                        if t == 8:
                            b2 = prep_b_load(2)
                        elif t == 11:
                            prep_b_finish(2, b2)
                        elif t == 12:
                            b3 = prep_b_load(3)
                        elif t == 15:
                            prep_b_finish(3, b3)
                    elif p < 3:
                        if t == 0:
                            b4 = prep_b_load(2 * p + 2)
                        elif t == 3:
                            prep_b_finish(2 * p + 2, b4)
                        elif t == 8:
                            b5 = prep_b_load(2 * p + 3)
                        elif t == 11:
                            prep_b_finish(2 * p + 3, b5)
                    ps2 = mpsum.tile([_P, 2, _NS], f32, tag="ps2")
                    for h in range(2):
                        mm_ts(t, 2 * p + h, ps2[:, h])
                    tp = t // 2
                    evac(ostP[tp][:, t % 2, :], ps2[:], t)
                    nc.sync.dma_start(
                        out_d[t * _P:(t + 1) * _P,
                              2 * p * _NS:(2 * p + 2) * _NS],
                        ostP[tp][:, t % 2, :],
                    )

    nc.compile()
    return nc


def _get_nc(normalize: bool):
    key = bool(normalize)
    if key not in _NC_CACHE:
        _NC_CACHE[key] = _build_nc(key)
    return _NC_CACHE[key]


def _pack_pairs(x16):
    """[R, 512] f16 row-major -> [R/2, 1024] where row tp || tp's pair:
    out[tp*128 + p] = x[tp*256 + p] ++ x[tp*256 + 128 + p]."""
    r = x16.shape[0]
    return (x16.reshape(r // 256, 2, _P, _D)
            .transpose(0, 2, 1, 3)
            .reshape(r // 2, 2 * _D))


def kernel(first_vector, second_vector, normalize):
    global LAST_RESULTS
    from concourse.bass_utils import run_bass_kernel_spmd

    a = np.asarray(first_vector, dtype=np.float32).astype(np.float16)
    b = np.asarray(second_vector, dtype=np.float32).astype(np.float16)
    assert a.shape == (_N, _D) and b.shape == (_M, _D)
    norm = bool(int(np.asarray(normalize)))

    nc = _get_nc(norm)

    in_maps = []
    for c in range(_GRID_N * _GRID_M):
        ni, mj = divmod(c, _GRID_M)
        in_maps.append(
            {
                "a": _pack_pairs(a[ni * _AN:(ni + 1) * _AN]),
                "b": _pack_pairs(b[mj * _BM:(mj + 1) * _BM]),
            }
        )

    res = run_bass_kernel_spmd(
        nc, in_maps, core_ids=list(range(_GRID_N * _GRID_M)), trace=TRACE
    )
    LAST_RESULTS = res

    out = np.empty((_N, _M), dtype=np.float32)
    for c in range(_GRID_N * _GRID_M):
        ni, mj = divmod(c, _GRID_M)
        out[ni * _AN:(ni + 1) * _AN, mj * _BM:(mj + 1) * _BM] = \
            res.results[c]["out"].astype(np.float32)
    return out
